# revision 4
# baseline (speedup 1.0000x reference)
"""Trainium2 Bass kernel for nn_AttentionWithBias_57406532878972.

Contract: kernel(**inputs) takes FULL (unsharded) numpy inputs (as produced
by the problem's setup_inputs) and returns the FULL output [2, 512, 256].

Sharding strategy: the q dimension (512) is split across the 8 NeuronCores
(64 q rows per batch per core). This partitions the dominant input (bias,
[2,512,512,128] fp32 = 256 MB) perfectly across cores with no duplicated
HBM traffic and no collectives: each core computes the full attention for
its q rows (it needs full K/V, which are cheap to compute redundantly) and
writes its slice of the output.

Per-core algorithm highlights:
  - The layernorm over bias's last dim (128) is folded into the bias->head
    projection: pair = (bias @ W')*rstd - (rowsum*rstd/128)*colsum(W') + beta@Wb
    where W' = b_gamma * Wb.  Only per-row mean/var are computed from bias
    (single bn_stats pass on DVE); the normalized bias tensor is never
    materialized.
  - bias tiles are transposed on the TensorEngine (matmul-by-identity) so the
    128-dim contraction lands on partitions; the rowsum rides as a ones
    column appended to W'.
  - Attention runs k-major ([k, q] score tiles): softmax needs no
    cross-partition reduction, exp skips max-subtraction (logits are O(1) by
    construction), the denominator rides as a ones column appended to V, and
    the exp output feeds attn@V directly as the stationary operand.
"""

import sys

sys.path.insert(0, "/opt/trn_rl_repo")

import numpy as np

import concourse.bass as bass
import concourse.mybir as mybir
import concourse.tile as tile
from concourse import bacc
from concourse.bass_utils import run_bass_kernel_spmd

F32 = mybir.dt.float32
AF = mybir.ActivationFunctionType
OP = mybir.AluOpType

B = 2
L = 512
DG = 256
DB = 128
H = 8
D = 32
EPS = 1e-5
NCORES = 8
QS = L // NCORES     # 64 q rows per batch per core
NSLAB = B * QS       # 128 slabs/core, slab s = (b, q)
KSCALE = float(1.0 / np.sqrt(D))

SGRP = 8             # slabs per stats-math batch
PGRP = 8             # slabs per pair psum bank group


def build_program():
    nc = bacc.Bacc(None, target_bir_lowering=False)

    xq_d = nc.dram_tensor("xq", [128, DG], F32, kind="ExternalInput")
    xf_d = nc.dram_tensor("xf", [B * L, DG], F32, kind="ExternalInput")
    bias_d = nc.dram_tensor("biass", [B, QS, L, DB], F32, kind="ExternalInput")
    wq_d = nc.dram_tensor("Wq", [DG, DG], F32, kind="ExternalInput")
    wk_d = nc.dram_tensor("Wk", [DG, DG], F32, kind="ExternalInput")
    wv_d = nc.dram_tensor("Wv", [DG, DG], F32, kind="ExternalInput")
    wg_d = nc.dram_tensor("Wg", [DG, DG], F32, kind="ExternalInput")
    wout_d = nc.dram_tensor("Wout", [DG, DG], F32, kind="ExternalInput")
    wext_d = nc.dram_tensor("Wext", [DB, 9], F32, kind="ExternalInput")
    csn_d = nc.dram_tensor("csnrep", [128, H], F32, kind="ExternalInput")
    cb_d = nc.dram_tensor("cBrep", [128, H], F32, kind="ExternalInput")
    gg_d = nc.dram_tensor("ggrep", [128, DG], F32, kind="ExternalInput")
    gb_d = nc.dram_tensor("gbrep", [128, DG], F32, kind="ExternalInput")
    bg_d = nc.dram_tensor("bgrep", [128, DG], F32, kind="ExternalInput")
    bo_d = nc.dram_tensor("boutrep", [128, DG], F32, kind="ExternalInput")
    out_d = nc.dram_tensor("outq", [128, DG], F32, kind="ExternalOutput")

    ident_d = nc.inline_tensor(np.eye(128, dtype=np.float32), name="ident")

    with tile.TileContext(nc) as tc:
        _body(nc, tc, xq_d, xf_d, bias_d, wq_d, wk_d, wv_d, wg_d,
              wout_d, wext_d, csn_d, cb_d, gg_d, gb_d, bg_d, bo_d,
              out_d, ident_d)
    nc.compile()
    return nc


def _body(nc, tc, xq_d, xf_d, bias_d, wq_d, wk_d, wv_d, wg_d, wout_d,
          wext_d, csn_d, cb_d, gg_d, gb_d, bg_d, bo_d, out_d, ident_d):
    import contextlib
    ctx = contextlib.ExitStack()
    with ctx:
        persist = ctx.enter_context(tc.tile_pool(name="persist", bufs=1))
        small = ctx.enter_context(tc.tile_pool(name="small", bufs=4))
        slabp = ctx.enter_context(tc.tile_pool(name="slabp", bufs=3))
        btp = ctx.enter_context(tc.tile_pool(name="btp", bufs=3))
        statsp = ctx.enter_context(tc.tile_pool(name="statsp", bufs=2))
        normp = ctx.enter_context(tc.tile_pool(name="normp", bufs=2))
        asmp = ctx.enter_context(tc.tile_pool(name="asmp", bufs=2))
        expp = ctx.enter_context(tc.tile_pool(name="expp", bufs=1))
        ps_t = ctx.enter_context(tc.tile_pool(name="ps_t", bufs=2, space="PSUM"))
        ps_pair = ctx.enter_context(
            tc.tile_pool(name="ps_pair", bufs=2, space="PSUM"))
        ps_sc = ctx.enter_context(tc.tile_pool(name="ps_sc", bufs=1, space="PSUM"))
        ps_av = ctx.enter_context(tc.tile_pool(name="ps_av", bufs=1, space="PSUM"))
        ps_s0 = ctx.enter_context(tc.tile_pool(name="ps_s0", bufs=1, space="PSUM"))

        # ---------------- constants / weights to SBUF ----------------
        ident = persist.tile([128, 128], F32)
        nc.sync.dma_start(ident[:], ident_d[:])
        wext = persist.tile([128, 9], F32)
        nc.sync.dma_start(wext[:], wext_d[:])
        csn = persist.tile([128, H], F32)
        nc.sync.dma_start(csn[:], csn_d[:])
        cbr = persist.tile([128, H], F32)
        nc.sync.dma_start(cbr[:], cb_d[:])
        ggr = persist.tile([128, DG], F32)
        nc.sync.dma_start(ggr[:], gg_d[:])
        gbr = persist.tile([128, DG], F32)
        nc.sync.dma_start(gbr[:], gb_d[:])
        bgr = persist.tile([128, DG], F32)
        nc.sync.dma_start(bgr[:], bg_d[:])
        bor = persist.tile([128, DG], F32)
        nc.sync.dma_start(bor[:], bo_d[:])
        wq = persist.tile([128, 2, DG], F32)
        nc.sync.dma_start(wq[:], wq_d.ap().rearrange("(c p) n -> p c n", p=128))
        wk = persist.tile([128, 2, DG], F32)
        nc.sync.dma_start(wk[:], wk_d.ap().rearrange("(c p) n -> p c n", p=128))
        wv = persist.tile([128, 2, DG], F32)
        nc.sync.dma_start(wv[:], wv_d.ap().rearrange("(c p) n -> p c n", p=128))
        wg = persist.tile([128, 2, DG], F32)
        nc.sync.dma_start(wg[:], wg_d.ap().rearrange("(c p) n -> p c n", p=128))
        wo = persist.tile([128, 2, DG], F32)
        nc.sync.dma_start(wo[:], wout_d.ap().rearrange("(c p) n -> p c n", p=128))
        epst = persist.tile([128, 1], F32)
        nc.vector.memset(epst[:], EPS)

        # ---------------- S0: x layernorm, transposes, projections --------
        def layernorm_tile(x_sb, n_par=128):
            st = small.tile([128, 6], F32, tag="lnst")
            nc.vector.bn_stats(st[:n_par], x_sb[:n_par])
            mv = small.tile([128, 2], F32, tag="lnmv")
            nc.vector.bn_aggr(mv[:n_par], st[:n_par])
            sd = small.tile([128, 1], F32, tag="lnsd")
            nc.scalar.activation(sd[:n_par], mv[:n_par, 1:2], AF.Sqrt,
                                 bias=epst[:n_par])
            rs = small.tile([128, 1], F32, tag="lnrs")
            nc.vector.reciprocal(rs[:n_par], sd[:n_par])
            nmr = small.tile([128, 1], F32, tag="lnnm")
            nc.vector.tensor_tensor(nmr[:n_par], mv[:n_par, 0:1], rs[:n_par],
                                    OP.mult)
            nc.vector.tensor_scalar_mul(nmr[:n_par], nmr[:n_par], -1.0)
            xc = small.tile([128, DG], F32, tag="lnxc")
            nc.scalar.activation(xc[:n_par], x_sb[:n_par], AF.Identity,
                                 bias=nmr[:n_par], scale=rs[:n_par])
            xn = small.tile([128, DG], F32, tag="lnxn")
            nc.vector.tensor_tensor(xn[:n_par], xc[:n_par], ggr[:n_par], OP.mult)
            nc.vector.tensor_tensor(xn[:n_par], xn[:n_par], gbr[:n_par], OP.add)
            return xn

        xnT = persist.tile([128, 2, B * L], F32)
        for i in range(8):
            xt = small.tile([128, DG], F32, tag="xload")
            nc.sync.dma_start(xt[:], xf_d[i * 128:(i + 1) * 128, :])
            xn = layernorm_tile(xt)
            pt = ps_s0.tile([128, 2, 128], F32, tag="xnt")
            for c in range(2):
                nc.tensor.transpose(pt[:, c, :], xn[:, c * 128:(c + 1) * 128],
                                    ident[:])
            for c in range(2):
                nc.scalar.copy(xnT[:, c, i * 128:(i + 1) * 128], pt[:, c, :])

        xnqT = persist.tile([128, 2, 128], F32)
        xqt = small.tile([128, DG], F32, tag="xload")
        nc.sync.dma_start(xqt[:], xq_d[:])
        xnq_keep = persist.tile([128, DG], F32)
        xnq = layernorm_tile(xqt)
        nc.vector.tensor_copy(xnq_keep[:], xnq[:])
        ptq = ps_s0.tile([128, 2, 128], F32, tag="xnt")
        for c in range(2):
            nc.tensor.transpose(ptq[:, c, :], xnq_keep[:, c * 128:(c + 1) * 128],
                                ident[:])
        for c in range(2):
            nc.scalar.copy(xnqT[:, c, :], ptq[:, c, :])

        # KT [hd-chunk][128, 1024], scaled by 1/sqrt(D)
        kT = persist.tile([128, 2, B * L], F32)
        for hc in range(2):
            for half in range(2):
                pk = ps_s0.tile([128, 512], F32, tag="proj")
                for gc in range(2):
                    nc.tensor.matmul(pk[:], wk[:, gc, hc * 128:(hc + 1) * 128],
                                     xnT[:, gc, half * 512:(half + 1) * 512],
                                     start=(gc == 0), stop=(gc == 1))
                nc.scalar.mul(kT[:, hc, half * 512:(half + 1) * 512], pk[:],
                              KSCALE)
        # QT [hd-chunk][128, 128]
        qT = persist.tile([128, 2, 128], F32)
        for hc in range(2):
            pq = ps_s0.tile([128, 512], F32, tag="proj")
            for gc in range(2):
                nc.tensor.matmul(pq[:, 0:128], wq[:, gc, hc * 128:(hc + 1) * 128],
                                 xnqT[:, gc, :], start=(gc == 0), stop=(gc == 1))
            nc.scalar.copy(qT[:, hc, :], pq[:, 0:128])

        # block-diagonal QT for batched per-head qk matmuls:
        # qbd[hc][b] is [128, 4*64]; rows hr*32..hr*32+32 hold QT of head
        # hc*4+hr in cols hr*64..hr*64+64, zero elsewhere.
        qbd = persist.tile([128, 2, 2, 4 * QS], F32)
        nc.vector.memset(qbd[:], 0.0)
        for hc in range(2):
            for b in range(B):
                for hr in range(4):
                    nc.vector.tensor_copy(
                        qbd[hr * 32:(hr + 1) * 32, hc, b,
                            hr * QS:(hr + 1) * QS],
                        qT[hr * 32:(hr + 1) * 32, hc, b * QS:(b + 1) * QS])

        # V_ext [tok-chunk][128, 8, 33] (col 32 of each h block = ones)
        vext = persist.tile([128, 8, H, D + 1], F32)
        nc.gpsimd.memset(vext[:], 1.0)
        for i in range(8):
            pv = ps_s0.tile([128, 512], F32, tag="proj")
            for gc in range(2):
                nc.tensor.matmul(pv[:, 0:256], xnT[:, gc, i * 128:(i + 1) * 128],
                                 wv[:, gc, :], start=(gc == 0), stop=(gc == 1))
            nc.scalar.copy(
                vext[:, i, :, 0:D],
                pv[:, 0:256].rearrange("p (h d) -> p h d", h=H))

        # gate = sigmoid(xnq @ Wg + bg)
        gate = persist.tile([128, DG], F32)
        pg = ps_s0.tile([128, 512], F32, tag="proj")
        for gc in range(2):
            nc.tensor.matmul(pg[:, 0:256], xnqT[:, gc, :], wg[:, gc, :],
                             start=(gc == 0), stop=(gc == 1))
        gpre = small.tile([128, DG], F32, tag="gpre")
        nc.vector.tensor_tensor(gpre[:], pg[:, 0:256], bgr[:], OP.add)
        nc.scalar.activation(gate[:], gpre[:], AF.Sigmoid)

        # ---------------- S1: bias slabs ----------------
        pair_norm = persist.tile([128, NSLAB, 4, H], F32)
        rstd_all = persist.tile([128, NSLAB, 4], F32)

        statsbuf = None
        pairbank = None
        for s in range(NSLAB):
            b, q = divmod(s, QS)
            si = s % SGRP
            pi = s % PGRP
            if si == 0:
                statsbuf = statsp.tile([128, SGRP * 4, 8], F32)
            if pi == 0:
                pairbank = ps_pair.tile([128, PGRP * 4 * 9], F32)

            a = slabp.tile([128, 4, DB + 4], F32)
            nc.sync.dma_start(
                a[:, :, 0:DB], bias_d[b, q].rearrange("(t p) c -> p t c", p=128))
            for t in range(4):
                nc.vector.bn_stats(statsbuf[:, si * 4 + t, 0:6],
                                   a[:, t, 0:DB])

            pt = ps_t.tile([128, 512], F32)
            for t in range(4):
                nc.tensor.transpose(pt[:, t * 128:(t + 1) * 128],
                                    a[:, t, 0:DB], ident[:])
            bT = btp.tile([128, 512], F32)
            nc.scalar.copy(bT[:], pt[:])
            for t in range(4):
                nc.tensor.matmul(
                    pairbank[:, (pi * 4 + t) * 9:(pi * 4 + t + 1) * 9],
                    bT[:, t * 128:(t + 1) * 128], wext[:],
                    start=True, stop=True)

            if si == SGRP - 1:
                g0 = s - (SGRP - 1)
                dq = statsp.tile([128, SGRP * 4], F32, tag="dq")
                nc.vector.tensor_tensor(dq[:], statsbuf[:, :, 1],
                                        statsbuf[:, :, 4], OP.subtract)
                nc.vector.tensor_scalar_mul(dq[:], dq[:], 0.5)
                cv = statsp.tile([128, SGRP * 4], F32, tag="cv")
                nc.vector.tensor_tensor(cv[:], statsbuf[:, :, 2],
                                        statsbuf[:, :, 5], OP.add)
                d2 = statsp.tile([128, SGRP * 4], F32, tag="d2")
                nc.vector.tensor_tensor(d2[:], dq[:], dq[:], OP.mult)
                var = statsp.tile([128, SGRP * 4], F32, tag="var")
                nc.vector.scalar_tensor_tensor(
                    out=var[:], in0=cv[:], scalar=1.0 / DB, in1=d2[:],
                    op0=OP.mult, op1=OP.add)
                sd = statsp.tile([128, SGRP * 4], F32, tag="sd")
                nc.scalar.activation(sd[:], var[:], AF.Sqrt, bias=epst[:])
                nc.vector.reciprocal(
                    rstd_all[:, g0:g0 + SGRP, :].rearrange("p a b -> p (a b)"),
                    sd[:])

            if pi == PGRP - 1:
                g0 = s - (PGRP - 1)
                pb = pairbank[:].rearrange("p (s t h) -> p s t h", s=PGRP, t=4)
                rsl = rstd_all[:, g0:g0 + PGRP, :]
                t1 = normp.tile([128, PGRP, 4, H], F32, tag="t1")
                nc.vector.tensor_tensor(t1[:], pb[:, :, :, 0:8],
                                        rsl.to_broadcast([128, PGRP, 4, H]),
                                        OP.mult)
                u = normp.tile([128, PGRP, 4], F32, tag="u")
                nc.vector.tensor_tensor(u[:], pb[:, :, :, 8], rsl, OP.mult)
                t2 = normp.tile([128, PGRP, 4, H], F32, tag="t2")
                nc.vector.tensor_tensor(
                    t2[:], u[:].to_broadcast([128, PGRP, 4, H]),
                    csn[:].rearrange("p h -> p () () h").to_broadcast(
                        [128, PGRP, 4, H]), OP.mult)
                nc.vector.tensor_tensor(pair_norm[:, g0:g0 + PGRP, :, :],
                                        t1[:], t2[:], OP.subtract)

        # ---------------- S2: attention ----------------
        final_out = persist.tile([128, DG], F32)
        for b in range(B):
            o_ps = ps_av.tile([64, H * (D + 1)], F32)
            expT = []
            for t in range(4):
                asm = asmp.tile([128, H * QS], F32)
                for h in range(H):
                    src = pair_norm[:, b * QS:(b + 1) * QS, t, h]
                    nc.vector.tensor_scalar_add(
                        asm[:, h * QS:(h + 1) * QS], src, cbr[:, h:h + 1])
                sc_ps = ps_sc.tile([128, H * QS], F32)
                for hc in range(2):
                    nc.tensor.matmul(
                        sc_ps[:, hc * 4 * QS:(hc + 1) * 4 * QS],
                        kT[:, hc, b * L + t * 128:b * L + (t + 1) * 128],
                        qbd[:, hc, b, :], start=True, stop=True)
                sc = asmp.tile([128, H * QS], F32, tag="scsb")
                nc.vector.tensor_tensor(sc[:], sc_ps[:], asm[:], OP.add)
                ex = expp.tile([128, H * QS], F32, tag=f"exp{b}_{t}")
                nc.scalar.activation(ex[:], sc[:], AF.Exp)
                expT.append(ex)
            for h in range(H):
                for t in range(4):
                    nc.tensor.matmul(
                        o_ps[:, h * (D + 1):(h + 1) * (D + 1)],
                        expT[t][:, h * QS:(h + 1) * QS],
                        vext[:, b * 4 + t, h, :],
                        start=(t == 0), stop=(t == 3))
            rp = small.tile([64, H], F32, tag="rp")
            nc.vector.reciprocal(
                rp[:], o_ps[:].rearrange("p (h e) -> p h e", h=H)[:, :, D])
            for h in range(H):
                nc.vector.tensor_tensor(
                    final_out[b * 64:(b + 1) * 64, h * D:(h + 1) * D],
                    o_ps[:, h * (D + 1):h * (D + 1) + D],
                    rp[:, h:h + 1].to_broadcast([64, D]), OP.mult)

        gout = persist.tile([128, DG], F32)
        nc.vector.tensor_tensor(gout[:], final_out[:], gate[:], OP.mult)
        pgt = ps_s0.tile([128, 2, 128], F32, tag="xnt")
        for c in range(2):
            nc.tensor.transpose(pgt[:, c, :], gout[:, c * 128:(c + 1) * 128],
                                ident[:])
        goT = small.tile([128, 2, 128], F32, tag="goT")
        for c in range(2):
            nc.scalar.copy(goT[:, c, :], pgt[:, c, :])
        pf = ps_s0.tile([128, 512], F32, tag="proj")
        for c in range(2):
            nc.tensor.matmul(pf[:, 0:256], goT[:, c, :], wo[:, c, :],
                             start=(c == 0), stop=(c == 1))
        fin = small.tile([128, DG], F32, tag="fin")
        nc.vector.tensor_tensor(fin[:], pf[:, 0:256], bor[:], OP.add)
        nc.sync.dma_start(out_d[:], fin[:])


def host_inputs(inputs, core):
    x = np.asarray(inputs["x"], np.float32)
    bias = np.asarray(inputs["bias"], np.float32)
    qs, qe = core * QS, (core + 1) * QS
    wb = np.asarray(inputs["Wb"], np.float32)
    b_gamma = np.asarray(inputs["b_gamma"], np.float32)
    b_beta = np.asarray(inputs["b_beta"], np.float32)
    wprime = b_gamma[:, None] * wb
    wext = np.concatenate([wprime, np.ones((DB, 1), np.float32)], 1)
    csn = wprime.sum(0) / DB
    cB = b_beta @ wb
    rep = lambda v: np.tile(np.asarray(v, np.float32)[None, :], (128, 1))
    return {
        "xq": np.ascontiguousarray(x[:, qs:qe, :]).reshape(128, DG),
        "xf": np.ascontiguousarray(x).reshape(B * L, DG),
        "biass": np.ascontiguousarray(bias[:, qs:qe, :, :]),
        "Wq": np.asarray(inputs["Wq"], np.float32),
        "Wk": np.asarray(inputs["Wk"], np.float32),
        "Wv": np.asarray(inputs["Wv"], np.float32),
        "Wg": np.asarray(inputs["Wg"], np.float32),
        "Wout": np.asarray(inputs["Wout"], np.float32),
        "Wext": wext,
        "csnrep": rep(csn),
        "cBrep": rep(cB),
        "ggrep": rep(inputs["g_gamma"]),
        "gbrep": rep(inputs["g_beta"]),
        "bgrep": rep(inputs["bg"]),
        "boutrep": rep(inputs["bout"]),
    }


_CACHED_NC = None


def get_program():
    global _CACHED_NC
    if _CACHED_NC is None:
        _CACHED_NC = build_program()
    return _CACHED_NC


def run_on_hw(inputs):
    nc = get_program()
    in_maps = [host_inputs(inputs, c) for c in range(NCORES)]
    res = run_bass_kernel_spmd(nc, in_maps, list(range(NCORES)))
    out = np.zeros((B, L, DG), np.float32)
    for c in range(NCORES):
        out[:, c * QS:(c + 1) * QS, :] = \
            res.results[c]["outq"].reshape(B, QS, DG)
    return out


def kernel(**inputs) -> np.ndarray:
    return run_on_hw(inputs)
